# revision 2
# baseline (speedup 1.0000x reference)
"""DetContrastiveLoss Trainium2 kernel.

Two SPMD phases over 8 NeuronCores (no ncfw collectives — their entry
barrier + launch skew costs more than the 1MB exchange itself):

  Host prep: transpose each BEV plane to channels-last [H*W, C] so one
    box's 256 channel values are contiguous (4KB aligned rows). Pure
    layout prep — no box information used.

  Phase A (per core k): own 128 boxes of batch b=k//2. Compute box pixel
    index r = cy*W+cx on-device (exact f32 chain matching the CPU-jax
    reference truncation semantics), one dma_gather of 128 windows of
    4 pixels x 256 channels (4KB) at row r//4 (fits the int16 index
    limit: r//4 <= 32399), select the r%4 pixel with 4 masked adds,
    L2-normalize rows (1/sqrt(temperature) folded in), transpose on
    PE -> fnT block [256, 128].

  Host: concat blocks -> fnT [256, 1024]; compute (state, class) atom
    per box from gt_boxes, sort columns by atom so per-atom masked
    maxima become segment maxima over static column slices. Group
    sizes are baked into the phase-B program at (lazy) compile time.

  Phase B (per core k): sim block [128, 1024] = own_fnT.T @ fnT_sorted
    via PE (f32), 6 segment column maxima, hinge, anchor-masked column
    sums via PE -> [1, 8] per core.

  Host: assemble the scalar loss from 8x6 partials + atom counts
  (f32 arithmetic mirroring the reference).
"""

import sys

for _p in ("/opt/trn_rl_repo", "/root/.axon_site/_ro/trn_rl_repo"):
    if _p not in sys.path:
        sys.path.append(_p)

import numpy as np

import concourse.bass as bass
import concourse.bacc as bacc
import concourse.tile as tile
import concourse.mybir as mybir
from concourse import bass_utils
from concourse.masks import make_identity

F32 = mybir.dt.float32
I32 = mybir.dt.int32
I16 = mybir.dt.int16

B, N, C, H, W = 4, 256, 256, 360, 360
HW = H * W            # 129600
M = B * N             # 1024
NCORES = 8
BOX = 128             # boxes per core
TEMPERATURE = 0.1
MARGIN = 0.2
X0 = -59.9
SPAN = 119.8
PIX = 4               # pixels per gather window
WIN = PIX * C         # floats per gather window (4KB)
NROWS = HW // PIX     # 32400 gather rows, fits int16
SQRT_INV_T = float(np.sqrt(np.float32(1.0) / np.float32(TEMPERATURE)))

AX = mybir.AxisListType
ALU = mybir.AluOpType


def _coord_chain(nc, pool, shape, src_ap, w_dim, tag):
    """clip((x - X0)/SPAN * w_dim, 0, w_dim-1) then floor -> float tile.

    Matches jnp on CPU: sub, div, mul, clip, trunc. Floor is computed as
    int-cast + cast-back + fix so it is exact under any HW cast rounding.
    """
    t = pool.tile(shape, F32, tag=f"{tag}_t")
    inv_span = float(np.float32(1.0) / np.float32(SPAN))
    nc.vector.tensor_scalar(out=t[:], in0=src_ap, scalar1=float(X0), scalar2=inv_span, op0=ALU.subtract, op1=ALU.mult)
    nc.vector.tensor_scalar(out=t[:], in0=t[:], scalar1=float(w_dim), scalar2=None, op0=ALU.mult)
    nc.vector.tensor_scalar(out=t[:], in0=t[:], scalar1=0.0, scalar2=float(w_dim - 1), op0=ALU.max, op1=ALU.min)
    return _floor(nc, pool, shape, t, tag)


def _floor(nc, pool, shape, t, tag):
    ti = pool.tile(shape, I32, tag=f"{tag}_i")
    nc.vector.tensor_copy(out=ti[:], in_=t[:])
    tb = pool.tile(shape, F32, tag=f"{tag}_b")
    nc.vector.tensor_copy(out=tb[:], in_=ti[:])
    gt = pool.tile(shape, F32, tag=f"{tag}_g")
    nc.vector.tensor_tensor(out=gt[:], in0=tb[:], in1=t[:], op=ALU.is_gt)
    fl = pool.tile(shape, F32, tag=f"{tag}_f")
    nc.vector.tensor_tensor(out=fl[:], in0=tb[:], in1=gt[:], op=ALU.subtract)
    return fl


def _rowcode(nc, pool, shape, bx_ap_x, bx_ap_y, tag):
    """cx, cy -> r = cy*W + cx, g = floor(r/4) (f32), o = r - 4g."""
    cx = _coord_chain(nc, pool, shape, bx_ap_x, W, f"cx{tag}")
    cy = _coord_chain(nc, pool, shape, bx_ap_y, H, f"cy{tag}")
    r = pool.tile(shape, F32, tag=f"r{tag}")
    nc.vector.tensor_scalar(out=r[:], in0=cy[:], scalar1=float(W), scalar2=None, op0=ALU.mult)
    nc.vector.tensor_tensor(out=r[:], in0=r[:], in1=cx[:], op=ALU.add)
    g_pre = pool.tile(shape, F32, tag=f"gp{tag}")
    nc.vector.tensor_scalar(out=g_pre[:], in0=r[:], scalar1=0.25, scalar2=None, op0=ALU.mult)
    g = _floor(nc, pool, shape, g_pre, f"g{tag}")
    return r, g


def build_phase_a():
    nc = bacc.Bacc("TRN2", target_bir_lowering=False, debug=False, num_devices=NCORES)
    spatial = nc.dram_tensor("spatial", [HW * C], F32, kind="ExternalInput")  # channels-last
    boxes = nc.dram_tensor("boxes", [BOX, 9], F32, kind="ExternalInput")
    fnt_out = nc.dram_tensor("fnt", [C, BOX], F32, kind="ExternalOutput")

    with tile.TileContext(nc) as tc:
        with tc.tile_pool(name="sb", bufs=1) as pool, \
             tc.tile_pool(name="ps", bufs=2, space="PSUM") as psp:
            # ---- load boxes in two layouts ----
            bx = pool.tile([BOX, 9], F32)
            nc.sync.dma_start(out=bx[:], in_=boxes.ap())
            bx16 = pool.tile([16, 8, 9], F32)
            nc.sync.dma_start(
                out=bx16[:],
                in_=boxes.ap().rearrange("(j q) f -> q j f", q=16),
            )

            # ---- [16, 8] wrapped layout: gather row ids -> int16, replicated ----
            _, g16 = _rowcode(nc, pool, [16, 8, 1], bx16[:, :, 0:1], bx16[:, :, 1:2], "16")
            idx16 = pool.tile([128, 8], I16)
            nc.vector.tensor_copy(out=idx16[:16], in_=g16[:].rearrange("q j f -> q (j f)"))
            for grp in range(1, 8):
                nc.sync.dma_start(out=idx16[16 * grp:16 * (grp + 1)], in_=idx16[:16])

            # ---- [128] per-box layout: within-window pixel offset o ----
            r, g = _rowcode(nc, pool, [BOX, 1], bx[:, 0:1], bx[:, 1:2], "")
            o = pool.tile([BOX, 1], F32)
            nc.vector.tensor_scalar(out=o[:], in0=g[:], scalar1=-float(PIX), scalar2=None, op0=ALU.mult)
            nc.vector.tensor_tensor(out=o[:], in0=r[:], in1=o[:], op=ALU.add)

            # ---- one gather: 128 windows of 4KB ----
            win = pool.tile([128, 1, WIN], F32)
            nc.gpsimd.dma_gather(
                out_ap=win[:],
                in_ap=spatial.ap().rearrange("(r e) -> r e", e=WIN),
                idxs_ap=idx16[:],
                num_idxs=BOX,
                num_idxs_reg=BOX,
                elem_size=WIN,
                single_packet=False,
            )

            # ---- select the r%4 pixel: feats = sum_q win[:, q*C:(q+1)*C] * (o==q) ----
            feats = pool.tile([BOX, C], F32)
            tmp = pool.tile([BOX, C], F32)
            for q in range(PIX):
                eq = pool.tile([BOX, 1], F32, tag=f"eq{q}")
                nc.vector.tensor_scalar(out=eq[:], in0=o[:], scalar1=float(q), scalar2=None, op0=ALU.is_equal)
                tgt = feats if q == 0 else tmp
                nc.vector.tensor_scalar(out=tgt[:], in0=win[:, 0, q * C:(q + 1) * C], scalar1=eq[:], scalar2=None, op0=ALU.mult)
                if q > 0:
                    nc.vector.tensor_tensor(out=feats[:], in0=feats[:], in1=tmp[:], op=ALU.add)

            # ---- normalize rows; fold 1/sqrt(T) ----
            sq = pool.tile([BOX, C], F32)
            nc.vector.tensor_tensor(out=sq[:], in0=feats[:], in1=feats[:], op=ALU.mult)
            ssq = pool.tile([BOX, 1], F32)
            nc.vector.tensor_reduce(out=ssq[:], in_=sq[:], op=ALU.add, axis=AX.X)
            nc.vector.tensor_scalar(out=ssq[:], in0=ssq[:], scalar1=1e-24, scalar2=None, op0=ALU.max)
            rt = pool.tile([BOX, 1], F32)
            nc.vector.reciprocal(out=rt[:], in_=ssq[:])          # 1/ssq
            nc.scalar.activation(rt[:], rt[:], mybir.ActivationFunctionType.Sqrt)  # ~1/norm
            # one Newton step on r ~= rsqrt(ssq): r' = r*(1.5 - 0.5*ssq*r^2)
            r2 = pool.tile([BOX, 1], F32)
            nc.vector.tensor_tensor(out=r2[:], in0=rt[:], in1=rt[:], op=ALU.mult)
            nc.vector.tensor_tensor(out=r2[:], in0=r2[:], in1=ssq[:], op=ALU.mult)
            nc.vector.tensor_scalar(out=r2[:], in0=r2[:], scalar1=-0.5, scalar2=1.5, op0=ALU.mult, op1=ALU.add)
            nc.vector.tensor_tensor(out=rt[:], in0=rt[:], in1=r2[:], op=ALU.mult)
            nc.vector.tensor_scalar(out=rt[:], in0=rt[:], scalar1=SQRT_INV_T, scalar2=None, op0=ALU.mult)
            fn = pool.tile([BOX, C], F32)
            nc.vector.tensor_scalar(out=fn[:], in0=feats[:], scalar1=rt[:], scalar2=None, op0=ALU.mult)

            # ---- transpose [128, 256] -> [256, 128] via PE ----
            ident = pool.tile([128, 128], F32)
            make_identity(nc, ident[:])
            fnt_sb = pool.tile([128, 2, 128], F32)
            for hh in range(2):
                pst = psp.tile([128, 128], F32, tag="pst")
                nc.tensor.transpose(out=pst[:], in_=fn[:, hh * 128:(hh + 1) * 128], identity=ident[:])
                nc.vector.tensor_copy(out=fnt_sb[:, hh, :], in_=pst[:])
            nc.sync.dma_start(
                out=fnt_out.ap().rearrange("(h c) b -> c h b", h=2),
                in_=fnt_sb[:],
            )
    nc.compile()
    return nc


def build_phase_b(sizes):
    """sizes: tuple of 6 ints (sorted atom group sizes, sum == M)."""
    offs = [0] * 6
    for a in range(1, 6):
        offs[a] = offs[a - 1] + sizes[a - 1]

    nc = bacc.Bacc("TRN2", target_bir_lowering=False, debug=False, num_devices=NCORES)
    fnt_all = nc.dram_tensor("fnt_all", [C, M], F32, kind="ExternalInput")
    own_fnt = nc.dram_tensor("own_fnt", [C, BOX], F32, kind="ExternalInput")
    oanchor = nc.dram_tensor("oanchor", [BOX, 6], F32, kind="ExternalInput")
    out = nc.dram_tensor("out", [1, 8], F32, kind="ExternalOutput")

    NEG = -1.0e9

    with tile.TileContext(nc) as tc:
        with tc.tile_pool(name="sb", bufs=1) as pool, \
             tc.tile_pool(name="ps1", bufs=1, space="PSUM") as psp1, \
             tc.tile_pool(name="ps", bufs=2, space="PSUM") as psp:
            # ---- load fnT (channel-major), 4 chunks for queue parallelism ----
            rhs = pool.tile([128, 2, M], F32)
            rhs_src = fnt_all.ap().rearrange("(h c) j -> c h j", h=2)
            for hh in range(2):
                for cb in range(2):
                    cols = slice(cb * 512, (cb + 1) * 512)
                    nc.sync.dma_start(out=rhs[:, hh, cols], in_=rhs_src[:, hh, cols])
            lhs = pool.tile([128, 2, BOX], F32)
            nc.sync.dma_start(out=lhs[:], in_=own_fnt.ap().rearrange("(h c) b -> c h b", h=2))
            oanc = pool.tile([BOX, 6], F32)
            nc.sync.dma_start(out=oanc[:], in_=oanchor.ap())

            # ---- sim block [128, 1024] in PSUM (2 banks) ----
            sim = psp1.tile([128, M], F32)
            for nb in range(2):
                cols = slice(nb * 512, (nb + 1) * 512)
                for hh in range(2):
                    nc.tensor.matmul(
                        out=sim[:, cols],
                        lhsT=lhs[:, hh, :],
                        rhs=rhs[:, hh, cols],
                        start=(hh == 0),
                        stop=(hh == 1),
                    )

            # ---- segment maxima per atom over sorted columns ----
            amax = pool.tile([BOX, 6], F32)
            nc.vector.memset(amax[:], NEG)
            for a in range(6):
                if sizes[a] > 0:
                    nc.vector.tensor_reduce(
                        out=amax[:, a:a + 1],
                        in_=sim[:, offs[a]:offs[a] + sizes[a]],
                        op=ALU.max, axis=AX.X,
                    )

            # ---- hinge per group, anchor-masked ----
            rhs6 = pool.tile([BOX, 6], F32)
            for g in range(6):
                s_c = 0 if g >= 3 else 1          # opposite-state block
                c = g % 3
                a_pos = s_c * 3 + c
                n1 = s_c * 3 + (c + 1) % 3
                n2 = s_c * 3 + (c + 2) % 3
                mn = pool.tile([BOX, 1], F32, tag="mn")
                nc.vector.tensor_tensor(out=mn[:], in0=amax[:, n1:n1 + 1], in1=amax[:, n2:n2 + 1], op=ALU.max)
                nc.vector.tensor_scalar(out=mn[:], in0=mn[:], scalar1=float(MARGIN), scalar2=None, op0=ALU.add)
                nc.vector.tensor_tensor(out=mn[:], in0=mn[:], in1=amax[:, a_pos:a_pos + 1], op=ALU.subtract)
                nc.vector.tensor_scalar(out=mn[:], in0=mn[:], scalar1=0.0, scalar2=None, op0=ALU.max)
                nc.vector.tensor_tensor(out=rhs6[:, g:g + 1], in0=mn[:], in1=oanc[:, g:g + 1], op=ALU.mult)

            ones = pool.tile([BOX, 1], F32)
            nc.vector.memset(ones[:], 1.0)
            psum_out = psp.tile([1, 6], F32, tag="po")
            nc.tensor.matmul(out=psum_out[:], lhsT=ones[:], rhs=rhs6[:], start=True, stop=True)
            osb = pool.tile([1, 8], F32)
            nc.vector.memset(osb[:], 0.0)
            nc.vector.tensor_copy(out=osb[:, 0:6], in_=psum_out[:])
            nc.sync.dma_start(out=out.ap(), in_=osb[:])
    nc.compile()
    return nc


_CACHE = {}
LAST_RESULTS = []   # [(phase, BassKernelResults), ...] of the most recent kernel() call


def _get_phase_a():
    if "a" not in _CACHE:
        _CACHE["a"] = build_phase_a()
    return _CACHE["a"]


def _get_phase_b(sizes):
    key = ("b", sizes)
    if key not in _CACHE:
        _CACHE[key] = build_phase_b(sizes)
    return _CACHE[key]


def kernel(spatial_features_2d: np.ndarray, gt_boxes: np.ndarray) -> np.ndarray:
    spatial = np.ascontiguousarray(spatial_features_2d, dtype=np.float32)
    boxes = np.ascontiguousarray(gt_boxes, dtype=np.float32)
    LAST_RESULTS.clear()

    # ---- host prep: channels-last planes (no box info used) ----
    planes = [np.ascontiguousarray(spatial[b].reshape(C, HW).T).reshape(-1) for b in range(B)]

    # ---- phase A: gather + normalize + transpose, data-parallel over boxes ----
    nca = _get_phase_a()
    in_a = []
    for k in range(NCORES):
        b = k // 2
        n0 = (k % 2) * BOX
        in_a.append({
            "spatial": planes[b],
            "boxes": boxes[b, n0:n0 + BOX, :],
        })
    res_a = bass_utils.run_bass_kernel_spmd(nca, in_a, core_ids=list(range(NCORES)))
    LAST_RESULTS.append(("a", res_a))
    blocks = [res_a.results[k]["fnt"] for k in range(NCORES)]       # each [C, BOX]
    fnt_all = np.concatenate(blocks, axis=1)                        # [C, M]

    # ---- host: atom per box, sort columns by atom ----
    flag = boxes[..., 7].reshape(M)
    cls = boxes[..., 8].astype(np.int32).reshape(M)
    dyn = flag != 0
    atom = np.where(dyn, cls, 3 + cls).astype(np.int64)             # 0-2 dyn, 3-5 static
    perm = np.argsort(atom, kind="stable")
    sizes = tuple(int(x) for x in np.bincount(atom, minlength=6))
    fnt_sorted = np.ascontiguousarray(fnt_all[:, perm])
    atom_sorted = atom[perm]
    oanchor_full = (atom_sorted[:, None] == np.arange(6)[None, :]).astype(np.float32)

    # ---- phase B: sim block + segment maxima + partials ----
    ncb = _get_phase_b(sizes)
    in_b = []
    for k in range(NCORES):
        sl = slice(k * BOX, (k + 1) * BOX)
        in_b.append({
            "fnt_all": fnt_sorted,
            "own_fnt": np.ascontiguousarray(fnt_sorted[:, sl]),
            "oanchor": np.ascontiguousarray(oanchor_full[sl]),
        })
    res_b = bass_utils.run_bass_kernel_spmd(ncb, in_b, core_ids=list(range(NCORES)))
    LAST_RESULTS.append(("b", res_b))
    parts = np.stack([res_b.results[k]["out"][0] for k in range(NCORES)])  # [8, 8]

    # ---- host: assemble the scalar loss (f32, mirrors the reference) ----
    f32 = np.float32
    psums = parts[:, 0:6].astype(np.float32).sum(axis=0, dtype=np.float32)  # [6]
    total = f32(0.0)
    cnt = f32(0.0)
    for g in range(6):
        s_c = 0 if g >= 3 else 1
        c = g % 3
        n_a = sizes[g]
        n_pos = sizes[s_c * 3 + c]
        n_neg = sizes[s_c * 3 + (c + 1) % 3] + sizes[s_c * 3 + (c + 2) % 3]
        if (n_a > 0) and (n_pos > 0) and (n_neg > 0):
            total = f32(total + f32(psums[g] / f32(max(n_a, 1))))
            cnt = f32(cnt + 1.0)
    loss = f32(total / max(cnt, f32(1.0))) if cnt > 0 else f32(0.0)
    return np.asarray(loss, dtype=np.float32)


# revision 9
# speedup vs baseline: 1.1383x; 1.1383x over previous
"""DetContrastiveLoss Trainium2 kernel.

Two SPMD phases over 8 NeuronCores (no ncfw collectives — their entry
barrier + launch skew costs more than the 1MB exchange itself):

  Host prep: transpose each BEV plane to channels-last [H*W, C] so one
    box's 256 channel values are contiguous (4KB aligned rows). Pure
    layout prep — no box information used.

  Phase A (per core k): own 128 boxes of batch b=k//2. Compute box pixel
    index r = cy*W+cx on-device (exact f32 chain matching the CPU-jax
    reference truncation semantics), gather the 4-pixel-aligned block
    row r//4 (fits the int16 index limit: r//4 <= 32399) as 4 parallel
    dma_gathers (one per pixel offset, one SWDGE queue each, 1KB
    windows), select the r%4 pixel with 4 masked adds, L2-normalize
    rows (1/sqrt(temperature) folded in), transpose on PE -> [256, 128].

  Host: concat blocks -> fnT [256, 1024]; compute (state, class) atom
    per box from gt_boxes, sort columns by atom so per-atom masked
    maxima become segment maxima over static column slices. Group
    sizes are baked into the phase-B program at (lazy) compile time.

  Phase B (per core k): sim block [128, 1024] = own_fnT.T @ fnT_sorted
    via PE (f32, two 512-col PSUM tiles so column-0 reductions overlap
    the column-1 matmuls), 6 segment column maxima, vectorized hinge,
    anchor-masked column sums via PE -> [1, 8] per core.

  Host: assemble the scalar loss from 8x6 partials + atom counts
  (f32 arithmetic mirroring the reference).
"""

import sys

for _p in ("/opt/trn_rl_repo", "/root/.axon_site/_ro/trn_rl_repo"):
    if _p not in sys.path:
        sys.path.append(_p)

import numpy as np

import concourse.bass as bass
import concourse.bacc as bacc
import concourse.tile as tile
import concourse.mybir as mybir
from concourse import bass_utils
from concourse.masks import make_identity

F32 = mybir.dt.float32
I32 = mybir.dt.int32
I16 = mybir.dt.int16

B, N, C, H, W = 4, 256, 256, 360, 360
HW = H * W            # 129600
M = B * N             # 1024
NCORES = 8
BOX = 128             # boxes per core
TEMPERATURE = 0.1
MARGIN = 0.2
X0 = -59.9
SPAN = 119.8
PIX = 4               # pixels per aligned block
WIN = PIX * C         # floats per block (4KB)
NROWS = HW // PIX     # 32400 gather rows, fits int16
SQRT_INV_T = float(np.sqrt(np.float32(1.0) / np.float32(TEMPERATURE)))
INV_SPAN = float(np.float32(1.0) / np.float32(SPAN))

AX = mybir.AxisListType
ALU = mybir.AluOpType


def _floor(nc, pool, shape, t, tag):
    """Exact floor of non-negative f32 under any HW cast rounding."""
    ti = pool.tile(shape, I32, tag=f"{tag}_i")
    nc.vector.tensor_copy(out=ti[:], in_=t[:])
    tb = pool.tile(shape, F32, tag=f"{tag}_b")
    nc.vector.tensor_copy(out=tb[:], in_=ti[:])
    gt = pool.tile(shape, F32, tag=f"{tag}_g")
    nc.vector.tensor_tensor(out=gt[:], in0=tb[:], in1=t[:], op=ALU.is_gt)
    fl = pool.tile(shape, F32, tag=f"{tag}_f")
    nc.vector.tensor_tensor(out=fl[:], in0=tb[:], in1=gt[:], op=ALU.subtract)
    return fl


def _fused_chain(nc, pool, pshape, src_ap, tag):
    """src [..., 2] = (x, y) -> r = cy*W + cx, g = floor(r/4), both [..., 1].

    Chain matches CPU-jax truncation semantics: sub, mult-by-recip, mult,
    clip, floor (W == H so both coords share one fused pass).
    """
    t = pool.tile(pshape, F32, tag=f"t{tag}")
    nc.vector.tensor_scalar(out=t[:], in0=src_ap, scalar1=float(X0), scalar2=INV_SPAN, op0=ALU.subtract, op1=ALU.mult)
    nc.vector.tensor_scalar(out=t[:], in0=t[:], scalar1=float(W), scalar2=0.0, op0=ALU.mult, op1=ALU.max)
    nc.vector.tensor_scalar(out=t[:], in0=t[:], scalar1=float(W - 1), scalar2=None, op0=ALU.min)
    fl = _floor(nc, pool, pshape, t, tag)
    rshape = pshape[:-1] + [1]
    r = pool.tile(rshape, F32, tag=f"r{tag}")
    nc.vector.tensor_scalar(out=r[:], in0=fl[:, :, 1:2], scalar1=float(W), scalar2=None, op0=ALU.mult)
    nc.vector.tensor_tensor(out=r[:], in0=r[:], in1=fl[:, :, 0:1], op=ALU.add)
    gp = pool.tile(rshape, F32, tag=f"gp{tag}")
    nc.vector.tensor_scalar(out=gp[:], in0=r[:], scalar1=0.25, scalar2=None, op0=ALU.mult)
    g = _floor(nc, pool, rshape, gp, f"g{tag}")
    return r, g


def build_phase_a():
    nc = bacc.Bacc("TRN2", target_bir_lowering=False, debug=False, num_devices=NCORES)
    spatial = nc.dram_tensor("spatial", [HW * C], F32, kind="ExternalInput")  # channels-last
    boxes = nc.dram_tensor("boxes", [BOX, 1, 2], F32, kind="ExternalInput")   # own xy
    boxes_rep = nc.dram_tensor("boxes_rep", [128, 8, 2], F32, kind="ExternalInput")  # wrapped+replicated xy
    fnt_out = nc.dram_tensor("fnt", [C, BOX], F32, kind="ExternalOutput")

    with tile.TileContext(nc) as tc:
        with tc.tile_pool(name="sb", bufs=1) as pool, \
             tc.tile_pool(name="ps", bufs=2, space="PSUM") as psp:
            # ---- load boxes: own layout (sync) + wrapped-replicated (scalar) ----
            bxy = pool.tile([BOX, 1, 2], F32)
            nc.sync.dma_start(out=bxy[:], in_=boxes.ap())
            brep = pool.tile([128, 8, 2], F32)
            nc.scalar.dma_start(out=brep[:], in_=boxes_rep.ap())

            # ---- wrapped layout: gather row ids -> int16 (already replicated) ----
            _, g16 = _fused_chain(nc, pool, [128, 8, 2], brep[:], "w")
            idx16 = pool.tile([128, 8], I16)
            nc.vector.tensor_copy(out=idx16[:], in_=g16[:].rearrange("p j f -> p (j f)"))

            # ---- own layout: within-block pixel offset o = r - 4g ----
            r1, g1 = _fused_chain(nc, pool, [BOX, 1, 2], bxy[:], "o")
            o = pool.tile([BOX, 1], F32)
            nc.vector.tensor_scalar(out=o[:], in0=g1[:].rearrange("p j f -> p (j f)"), scalar1=-float(PIX), scalar2=None, op0=ALU.mult)
            nc.vector.tensor_tensor(out=o[:], in0=r1[:].rearrange("p j f -> p (j f)"), in1=o[:], op=ALU.add)

            # ---- one gather: 128 windows of 4KB (4 pixels x 256 ch) ----
            win = pool.tile([128, 1, WIN], F32)
            nc.gpsimd.dma_gather(
                out_ap=win[:],
                in_ap=spatial.ap().rearrange("(r e) -> r e", e=WIN),
                idxs_ap=idx16[:],
                num_idxs=BOX,
                num_idxs_reg=BOX,
                elem_size=WIN,
                single_packet=False,
            )

            # ---- select the r%4 pixel: feats = sum_q win[:, q*C:(q+1)*C] * (o==q) ----
            feats = pool.tile([BOX, C], F32)
            tmp = pool.tile([BOX, C], F32)
            for q in range(PIX):
                eq = pool.tile([BOX, 1], F32, tag=f"eq{q}")
                nc.vector.tensor_scalar(out=eq[:], in0=o[:], scalar1=float(q), scalar2=None, op0=ALU.is_equal)
                tgt = feats if q == 0 else tmp
                nc.vector.tensor_scalar(out=tgt[:], in0=win[:, 0, q * C:(q + 1) * C], scalar1=eq[:], scalar2=None, op0=ALU.mult)
                if q > 0:
                    nc.vector.tensor_tensor(out=feats[:], in0=feats[:], in1=tmp[:], op=ALU.add)

            # ---- normalize rows; fold 1/sqrt(T) ----
            sq = pool.tile([BOX, C], F32)
            nc.vector.tensor_tensor(out=sq[:], in0=feats[:], in1=feats[:], op=ALU.mult)
            ssq = pool.tile([BOX, 1], F32)
            nc.vector.tensor_reduce(out=ssq[:], in_=sq[:], op=ALU.add, axis=AX.X)
            nc.vector.tensor_scalar(out=ssq[:], in0=ssq[:], scalar1=1e-24, scalar2=None, op0=ALU.max)
            rt = pool.tile([BOX, 1], F32)
            nc.vector.reciprocal(out=rt[:], in_=ssq[:])          # 1/ssq
            nc.scalar.activation(rt[:], rt[:], mybir.ActivationFunctionType.Sqrt)  # ~1/norm
            # one Newton step on r ~= rsqrt(ssq): r' = r*(1.5 - 0.5*ssq*r^2)
            r2 = pool.tile([BOX, 1], F32)
            nc.vector.tensor_tensor(out=r2[:], in0=rt[:], in1=rt[:], op=ALU.mult)
            nc.vector.tensor_tensor(out=r2[:], in0=r2[:], in1=ssq[:], op=ALU.mult)
            nc.vector.tensor_scalar(out=r2[:], in0=r2[:], scalar1=-0.5, scalar2=1.5, op0=ALU.mult, op1=ALU.add)
            nc.vector.tensor_tensor(out=rt[:], in0=rt[:], in1=r2[:], op=ALU.mult)
            nc.vector.tensor_scalar(out=rt[:], in0=rt[:], scalar1=SQRT_INV_T, scalar2=None, op0=ALU.mult)
            fn = pool.tile([BOX, C], F32)
            nc.vector.tensor_scalar(out=fn[:], in0=feats[:], scalar1=rt[:], scalar2=None, op0=ALU.mult)

            # ---- transpose [128, 256] -> [256, 128] via PE ----
            ident = pool.tile([128, 128], F32)
            make_identity(nc, ident[:])
            fnt_sb = pool.tile([128, 2, 128], F32)
            for hh in range(2):
                pst = psp.tile([128, 128], F32, tag="pst")
                nc.tensor.transpose(out=pst[:], in_=fn[:, hh * 128:(hh + 1) * 128], identity=ident[:])
                nc.vector.tensor_copy(out=fnt_sb[:, hh, :], in_=pst[:])
            nc.sync.dma_start(
                out=fnt_out.ap().rearrange("(h c) b -> c h b", h=2),
                in_=fnt_sb[:],
            )
    nc.compile()
    return nc


def build_phase_b(sizes):
    """sizes: tuple of 6 ints (sorted atom group sizes, sum == M)."""
    offs = [0] * 6
    for a in range(1, 6):
        offs[a] = offs[a - 1] + sizes[a - 1]

    nc = bacc.Bacc("TRN2", target_bir_lowering=False, debug=False, num_devices=NCORES)
    fnt_all = nc.dram_tensor("fnt_all", [C, M], F32, kind="ExternalInput")
    own_fnt = nc.dram_tensor("own_fnt", [C, BOX], F32, kind="ExternalInput")
    oanchor = nc.dram_tensor("oanchor", [BOX, 6], F32, kind="ExternalInput")
    out = nc.dram_tensor("out", [1, 8], F32, kind="ExternalOutput")

    NEG = -1.0e9
    HB = 512  # columns per PSUM sim tile

    with tile.TileContext(nc) as tc:
        with tc.tile_pool(name="sb", bufs=1) as pool, \
             tc.tile_pool(name="ps1", bufs=2, space="PSUM") as psp1, \
             tc.tile_pool(name="ps", bufs=2, space="PSUM") as psp:
            # ---- loads: lhs first; quadrants split across sync+scalar HWDGE ----
            lhs = pool.tile([128, 2, BOX], F32)
            nc.sync.dma_start(out=lhs[:], in_=own_fnt.ap().rearrange("(h c) b -> c h b", h=2))
            src = fnt_all.ap().rearrange("(h c) j -> c h j", h=2)
            quad = {}
            engs = [nc.scalar, nc.sync, nc.scalar, nc.sync]
            i = 0
            for nb in range(2):
                for hh in range(2):
                    qt = pool.tile([128, HB], F32, tag=f"quad{hh}{nb}")
                    engs[i].dma_start(out=qt[:], in_=src[:, hh, nb * HB:(nb + 1) * HB])
                    i += 1
                    quad[(hh, nb)] = qt
            oanc = pool.tile([BOX, 6], F32)
            nc.scalar.dma_start(out=oanc[:], in_=oanchor.ap())

            # ---- sim in two PSUM tiles so nb=0 reductions overlap nb=1 matmuls ----
            sims = []
            for nb in range(2):
                s = psp1.tile([128, HB], F32, tag=f"sim{nb}")
                for hh in range(2):
                    nc.tensor.matmul(
                        out=s[:],
                        lhsT=lhs[:, hh, :],
                        rhs=quad[(hh, nb)][:],
                        start=(hh == 0),
                        stop=(hh == 1),
                    )
                sims.append(s)

            # ---- segment maxima per atom over sorted columns ----
            amax = pool.tile([BOX, 2, 3], F32)   # [s, c] triples
            if any(sz == 0 for sz in sizes):
                nc.vector.memset(amax[:], NEG)
            # issue all nb=0 segment reductions first (they only wait on sims[0])
            strad = {}
            for nb in range(2):
                for a in range(6):
                    lo, hi = offs[a], offs[a] + sizes[a]
                    l, h = max(lo, nb * HB), min(hi, (nb + 1) * HB)
                    if l >= h:
                        continue
                    dst = amax[:, a // 3, a % 3:a % 3 + 1]
                    seg = sims[nb][:, l - nb * HB:h - nb * HB]
                    if nb == 1 and a in strad:
                        t2 = pool.tile([BOX, 1], F32, tag=f"str{a}")
                        nc.vector.tensor_reduce(out=t2[:], in_=seg, op=ALU.max, axis=AX.X)
                        nc.vector.tensor_tensor(out=dst, in0=dst, in1=t2[:], op=ALU.max)
                    else:
                        nc.vector.tensor_reduce(out=dst, in_=seg, op=ALU.max, axis=AX.X)
                        strad[a] = True

            # ---- vectorized hinge over all 6 groups ----
            # P[:, s, c] = amax[:, 1-s, c] (opposite-state pos atom)
            P = pool.tile([BOX, 2, 3], F32)
            nc.vector.tensor_copy(out=P[:, 0, :], in_=amax[:, 1, :])
            nc.vector.tensor_copy(out=P[:, 1, :], in_=amax[:, 0, :])
            # N1[:, s, c] = P[:, s, (c+1)%3]; N2[:, s, c] = P[:, s, (c+2)%3]
            N1 = pool.tile([BOX, 2, 3], F32)
            nc.vector.tensor_copy(out=N1[:, :, 0:2], in_=P[:, :, 1:3])
            nc.vector.tensor_copy(out=N1[:, :, 2:3], in_=P[:, :, 0:1])
            N2 = pool.tile([BOX, 2, 3], F32)
            nc.vector.tensor_copy(out=N2[:, :, 0:1], in_=P[:, :, 2:3])
            nc.vector.tensor_copy(out=N2[:, :, 1:3], in_=P[:, :, 0:2])
            # hinge = relu((MARGIN + max(N1, N2)) - P), matching reference op order
            hx = pool.tile([BOX, 2, 3], F32)
            nc.vector.tensor_tensor(out=hx[:], in0=N1[:], in1=N2[:], op=ALU.max)
            nc.vector.tensor_scalar(out=hx[:], in0=hx[:], scalar1=float(MARGIN), scalar2=None, op0=ALU.add)
            nc.vector.tensor_tensor(out=hx[:], in0=hx[:], in1=P[:], op=ALU.subtract)
            nc.vector.tensor_scalar(out=hx[:], in0=hx[:], scalar1=0.0, scalar2=None, op0=ALU.max)
            rhs6 = pool.tile([BOX, 6], F32)
            nc.vector.tensor_tensor(out=rhs6[:], in0=hx[:].rearrange("p s c -> p (s c)"), in1=oanc[:], op=ALU.mult)

            ones = pool.tile([BOX, 1], F32)
            nc.vector.memset(ones[:], 1.0)
            psum_out = psp.tile([1, 6], F32, tag="po")
            nc.tensor.matmul(out=psum_out[:], lhsT=ones[:], rhs=rhs6[:], start=True, stop=True)
            osb = pool.tile([1, 8], F32)
            nc.vector.memset(osb[:], 0.0)
            nc.vector.tensor_copy(out=osb[:, 0:6], in_=psum_out[:])
            nc.sync.dma_start(out=out.ap(), in_=osb[:])
    nc.compile()
    return nc


_CACHE = {}
LAST_RESULTS = []   # [(phase, BassKernelResults), ...] of the most recent kernel() call


def _get_phase_a():
    if "a" not in _CACHE:
        _CACHE["a"] = build_phase_a()
    return _CACHE["a"]


def _get_phase_b(sizes):
    key = ("b", sizes)
    if key not in _CACHE:
        _CACHE[key] = build_phase_b(sizes)
    return _CACHE[key]


def kernel(spatial_features_2d: np.ndarray, gt_boxes: np.ndarray) -> np.ndarray:
    spatial = np.ascontiguousarray(spatial_features_2d, dtype=np.float32)
    boxes = np.ascontiguousarray(gt_boxes, dtype=np.float32)
    LAST_RESULTS.clear()

    # ---- host prep: channels-last planes (no box info used) ----
    planes = [np.ascontiguousarray(spatial[b].reshape(C, HW).T).reshape(-1) for b in range(B)]

    # ---- phase A: gather + normalize + transpose, data-parallel over boxes ----
    nca = _get_phase_a()
    in_a = []
    for k in range(NCORES):
        b = k // 2
        n0 = (k % 2) * BOX
        own_xy = boxes[b, n0:n0 + BOX, 0:2]
        # wrapped (j, q) -> box j*16+q at partition q, replicated to 8 groups
        wrapped = np.ascontiguousarray(own_xy.reshape(8, 16, 2).transpose(1, 0, 2))  # [16, 8, 2]
        rep = np.tile(wrapped, (8, 1, 1)).reshape(128, 8, 2)
        in_a.append({
            "spatial": planes[b],
            "boxes": np.ascontiguousarray(own_xy.reshape(BOX, 1, 2)),
            "boxes_rep": np.ascontiguousarray(rep),
        })
    res_a = bass_utils.run_bass_kernel_spmd(nca, in_a, core_ids=list(range(NCORES)))
    LAST_RESULTS.append(("a", res_a))
    blocks = [res_a.results[k]["fnt"] for k in range(NCORES)]       # each [C, BOX]
    fnt_all = np.concatenate(blocks, axis=1)                        # [C, M]

    # ---- host: atom per box, sort columns by atom ----
    flag = boxes[..., 7].reshape(M)
    cls = boxes[..., 8].astype(np.int32).reshape(M)
    dyn = flag != 0
    atom = np.where(dyn, cls, 3 + cls).astype(np.int64)             # 0-2 dyn, 3-5 static
    perm = np.argsort(atom, kind="stable")
    sizes = tuple(int(x) for x in np.bincount(atom, minlength=6))
    fnt_sorted = np.ascontiguousarray(fnt_all[:, perm])
    atom_sorted = atom[perm]
    oanchor_full = (atom_sorted[:, None] == np.arange(6)[None, :]).astype(np.float32)

    # ---- phase B: sim block + segment maxima + partials ----
    ncb = _get_phase_b(sizes)
    in_b = []
    for k in range(NCORES):
        sl = slice(k * BOX, (k + 1) * BOX)
        in_b.append({
            "fnt_all": fnt_sorted,
            "own_fnt": np.ascontiguousarray(fnt_sorted[:, sl]),
            "oanchor": np.ascontiguousarray(oanchor_full[sl]),
        })
    res_b = bass_utils.run_bass_kernel_spmd(ncb, in_b, core_ids=list(range(NCORES)))
    LAST_RESULTS.append(("b", res_b))
    parts = np.stack([res_b.results[k]["out"][0] for k in range(NCORES)])  # [8, 8]

    # ---- host: assemble the scalar loss (f32, mirrors the reference) ----
    f32 = np.float32
    psums = parts[:, 0:6].astype(np.float32).sum(axis=0, dtype=np.float32)  # [6]
    total = f32(0.0)
    cnt = f32(0.0)
    for g in range(6):
        s_c = 0 if g >= 3 else 1
        c = g % 3
        n_a = sizes[g]
        n_pos = sizes[s_c * 3 + c]
        n_neg = sizes[s_c * 3 + (c + 1) % 3] + sizes[s_c * 3 + (c + 2) % 3]
        if (n_a > 0) and (n_pos > 0) and (n_neg > 0):
            total = f32(total + f32(psums[g] / f32(max(n_a, 1))))
            cnt = f32(cnt + 1.0)
    loss = f32(total / max(cnt, f32(1.0))) if cnt > 0 else f32(0.0)
    return np.asarray(loss, dtype=np.float32)


# revision 12
# speedup vs baseline: 1.1415x; 1.0028x over previous
"""DetContrastiveLoss Trainium2 kernel.

Two SPMD phases over 8 NeuronCores (no ncfw collectives — their entry
barrier + launch skew costs more than the 1MB exchange itself):

  Host prep: transpose each BEV plane to channels-last [H*W, C] so one
    box's 256 channel values are contiguous (4KB aligned rows). Pure
    layout prep — no box information used.

  Phase A (per core k): own 128 boxes of batch b=k//2. Compute box pixel
    index r = cy*W+cx on-device (exact f32 chain matching the CPU-jax
    reference truncation semantics), gather the 4-pixel-aligned block
    row r//4 (fits the int16 index limit: r//4 <= 32399) as 4 parallel
    dma_gathers (one per pixel offset, one SWDGE queue each, 1KB
    windows), select the r%4 pixel with 4 masked adds, L2-normalize
    rows (1/sqrt(temperature) folded in), transpose on PE -> [256, 128].

  Host: concat blocks -> fnT [256, 1024]; compute (state, class) atom
    per box from gt_boxes, sort columns by atom so per-atom masked
    maxima become segment maxima over static column slices. Group
    sizes are baked into the phase-B program at (lazy) compile time.

  Phase B (per core k): sim block [128, 1024] = own_fnT.T @ fnT_sorted
    via PE (f32, two 512-col PSUM tiles so column-0 reductions overlap
    the column-1 matmuls), 6 segment column maxima, vectorized hinge,
    anchor-masked column sums via PE -> [1, 8] per core.

  Host: assemble the scalar loss from 8x6 partials + atom counts
  (f32 arithmetic mirroring the reference).
"""

import sys

for _p in ("/opt/trn_rl_repo", "/root/.axon_site/_ro/trn_rl_repo"):
    if _p not in sys.path:
        sys.path.append(_p)

import numpy as np

import concourse.bass as bass
import concourse.bacc as bacc
import concourse.tile as tile
import concourse.mybir as mybir
from concourse import bass_utils
from concourse.masks import make_identity

F32 = mybir.dt.float32
I32 = mybir.dt.int32
I16 = mybir.dt.int16

B, N, C, H, W = 4, 256, 256, 360, 360
HW = H * W            # 129600
M = B * N             # 1024
NCORES = 8
BOX = 128             # boxes per core
TEMPERATURE = 0.1
MARGIN = 0.2
X0 = -59.9
SPAN = 119.8
PIX = 4               # pixels per aligned block
WIN = PIX * C         # floats per block (4KB)
NROWS = HW // PIX     # 32400 gather rows, fits int16
SQRT_INV_T = float(np.sqrt(np.float32(1.0) / np.float32(TEMPERATURE)))
INV_SPAN = float(np.float32(1.0) / np.float32(SPAN))

AX = mybir.AxisListType
ALU = mybir.AluOpType


def _floor(nc, pool, shape, t, tag):
    """Exact floor of non-negative f32 under any HW cast rounding."""
    ti = pool.tile(shape, I32, tag=f"{tag}_i")
    nc.vector.tensor_copy(out=ti[:], in_=t[:])
    tb = pool.tile(shape, F32, tag=f"{tag}_b")
    nc.vector.tensor_copy(out=tb[:], in_=ti[:])
    gt = pool.tile(shape, F32, tag=f"{tag}_g")
    nc.vector.tensor_tensor(out=gt[:], in0=tb[:], in1=t[:], op=ALU.is_gt)
    fl = pool.tile(shape, F32, tag=f"{tag}_f")
    nc.vector.tensor_tensor(out=fl[:], in0=tb[:], in1=gt[:], op=ALU.subtract)
    return fl


def _fused_chain(nc, pool, pshape, src_ap, tag):
    """src [..., 2] = (x, y) -> r = cy*W + cx, g = floor(r/4), both [..., 1].

    Chain matches CPU-jax truncation semantics: sub, mult-by-recip, mult,
    clip, floor (W == H so both coords share one fused pass).
    """
    t = pool.tile(pshape, F32, tag=f"t{tag}")
    nc.vector.tensor_scalar(out=t[:], in0=src_ap, scalar1=float(X0), scalar2=INV_SPAN, op0=ALU.subtract, op1=ALU.mult)
    nc.vector.tensor_scalar(out=t[:], in0=t[:], scalar1=float(W), scalar2=0.0, op0=ALU.mult, op1=ALU.max)
    nc.vector.tensor_scalar(out=t[:], in0=t[:], scalar1=float(W - 1), scalar2=None, op0=ALU.min)
    fl = _floor(nc, pool, pshape, t, tag)
    rshape = pshape[:-1] + [1]
    r = pool.tile(rshape, F32, tag=f"r{tag}")
    nc.vector.tensor_scalar(out=r[:], in0=fl[:, :, 1:2], scalar1=float(W), scalar2=None, op0=ALU.mult)
    nc.vector.tensor_tensor(out=r[:], in0=r[:], in1=fl[:, :, 0:1], op=ALU.add)
    gp = pool.tile(rshape, F32, tag=f"gp{tag}")
    nc.vector.tensor_scalar(out=gp[:], in0=r[:], scalar1=0.25, scalar2=None, op0=ALU.mult)
    g = _floor(nc, pool, rshape, gp, f"g{tag}")
    return r, g


def build_phase_a():
    nc = bacc.Bacc("TRN2", target_bir_lowering=False, debug=False, num_devices=NCORES)
    spatial = nc.dram_tensor("spatial", [HW * C], F32, kind="ExternalInput")  # channels-last
    boxes_rep = nc.dram_tensor("boxes_rep", [128, 8, 2], F32, kind="ExternalInput")  # wrapped+replicated xy
    consts = nc.dram_tensor("consts", [128, 12], F32, kind="ExternalInput")   # [dmask(8) | 0,1,2,3]
    fnt_out = nc.dram_tensor("fnt", [C, BOX], F32, kind="ExternalOutput")

    with tile.TileContext(nc) as tc:
        with tc.tile_pool(name="sb", bufs=1) as pool, \
             tc.tile_pool(name="ps", bufs=2, space="PSUM") as psp:
            # ---- load boxes (scalar) + host constants (sync) ----
            brep = pool.tile([128, 8, 2], F32)
            nc.scalar.dma_start(out=brep[:], in_=boxes_rep.ap())
            cst = pool.tile([128, 12], F32)
            nc.sync.dma_start(out=cst[:], in_=consts.ap())

            # ---- wrapped layout: gather row ids -> int16 (already replicated) ----
            r16, g16 = _fused_chain(nc, pool, [128, 8, 2], brep[:], "w")
            idx16 = pool.tile([128, 8], I16)
            nc.vector.tensor_copy(out=idx16[:], in_=g16[:].rearrange("p j f -> p (j f)"))

            # ---- within-block pixel offset o = r - 4g, extracted at the
            #      diagonal (partition p holds box p at slot j = p//16) ----
            o_rep = pool.tile([128, 8, 1], F32)
            nc.vector.tensor_scalar(out=o_rep[:], in0=g16[:], scalar1=-float(PIX), scalar2=None, op0=ALU.mult)
            nc.vector.tensor_tensor(out=o_rep[:], in0=r16[:], in1=o_rep[:], op=ALU.add)
            od = pool.tile([128, 8], F32)
            nc.vector.tensor_tensor(out=od[:], in0=o_rep[:].rearrange("p j f -> p (j f)"), in1=cst[:, 0:8], op=ALU.mult)
            o = pool.tile([BOX, 1], F32)
            nc.vector.tensor_reduce(out=o[:], in_=od[:], op=ALU.add, axis=AX.X)

            # ---- one gather: 128 windows of 4KB (4 pixels x 256 ch) ----
            win = pool.tile([128, 1, WIN], F32)
            nc.gpsimd.dma_gather(
                out_ap=win[:],
                in_ap=spatial.ap().rearrange("(r e) -> r e", e=WIN),
                idxs_ap=idx16[:],
                num_idxs=BOX,
                num_idxs_reg=BOX,
                elem_size=WIN,
                single_packet=False,
            )

            # ---- select the r%4 pixel: one-hot (o==q) mask, one mult + 3 adds ----
            eq4 = pool.tile([128, PIX, 1], F32)
            nc.vector.tensor_scalar(
                out=eq4[:],
                in0=cst[:, 8:12].rearrange("p (q f) -> p q f", f=1),
                scalar1=o[:], scalar2=None, op0=ALU.is_equal,
            )
            prod = pool.tile([128, PIX, C], F32)
            nc.vector.tensor_tensor(
                out=prod[:],
                in0=win[:, 0, :].rearrange("p (q c) -> p q c", q=PIX),
                in1=eq4[:].to_broadcast([128, PIX, C]),
                op=ALU.mult,
            )
            s01 = pool.tile([BOX, C], F32)
            nc.vector.tensor_tensor(out=s01[:], in0=prod[:, 0, :], in1=prod[:, 1, :], op=ALU.add)
            s23 = pool.tile([BOX, C], F32)
            nc.vector.tensor_tensor(out=s23[:], in0=prod[:, 2, :], in1=prod[:, 3, :], op=ALU.add)
            feats = pool.tile([BOX, C], F32)
            nc.vector.tensor_tensor(out=feats[:], in0=s01[:], in1=s23[:], op=ALU.add)

            # ---- normalize rows; fold 1/sqrt(T) ----
            sq = pool.tile([BOX, C], F32)
            nc.vector.tensor_tensor(out=sq[:], in0=feats[:], in1=feats[:], op=ALU.mult)
            ssq = pool.tile([BOX, 1], F32)
            nc.vector.tensor_reduce(out=ssq[:], in_=sq[:], op=ALU.add, axis=AX.X)
            nc.vector.tensor_scalar(out=ssq[:], in0=ssq[:], scalar1=1e-24, scalar2=None, op0=ALU.max)
            rt = pool.tile([BOX, 1], F32)
            nc.vector.reciprocal(out=rt[:], in_=ssq[:])          # 1/ssq
            nc.scalar.activation(rt[:], rt[:], mybir.ActivationFunctionType.Sqrt)  # ~1/norm
            # one Newton step on r ~= rsqrt(ssq): r' = r*(1.5 - 0.5*ssq*r^2)
            r2 = pool.tile([BOX, 1], F32)
            nc.vector.tensor_tensor(out=r2[:], in0=rt[:], in1=rt[:], op=ALU.mult)
            nc.vector.tensor_tensor(out=r2[:], in0=r2[:], in1=ssq[:], op=ALU.mult)
            nc.vector.tensor_scalar(out=r2[:], in0=r2[:], scalar1=-0.5, scalar2=1.5, op0=ALU.mult, op1=ALU.add)
            nc.vector.tensor_tensor(out=rt[:], in0=rt[:], in1=r2[:], op=ALU.mult)
            nc.vector.tensor_scalar(out=rt[:], in0=rt[:], scalar1=SQRT_INV_T, scalar2=None, op0=ALU.mult)
            fn = pool.tile([BOX, C], F32)
            nc.vector.tensor_scalar(out=fn[:], in0=feats[:], scalar1=rt[:], scalar2=None, op0=ALU.mult)

            # ---- transpose [128, 256] -> [256, 128] via PE ----
            ident = pool.tile([128, 128], F32)
            make_identity(nc, ident[:])
            fnt_sb = pool.tile([128, 2, 128], F32)
            for hh in range(2):
                pst = psp.tile([128, 128], F32, tag="pst")
                nc.tensor.transpose(out=pst[:], in_=fn[:, hh * 128:(hh + 1) * 128], identity=ident[:])
                nc.vector.tensor_copy(out=fnt_sb[:, hh, :], in_=pst[:])
            nc.sync.dma_start(
                out=fnt_out.ap().rearrange("(h c) b -> c h b", h=2),
                in_=fnt_sb[:],
            )
    nc.compile()
    return nc


def build_phase_b(sizes):
    """sizes: tuple of 6 ints (sorted atom group sizes, sum == M)."""
    offs = [0] * 6
    for a in range(1, 6):
        offs[a] = offs[a - 1] + sizes[a - 1]

    nc = bacc.Bacc("TRN2", target_bir_lowering=False, debug=False, num_devices=NCORES)
    fnt_all = nc.dram_tensor("fnt_all", [C, M], F32, kind="ExternalInput")
    own_fnt = nc.dram_tensor("own_fnt", [C, BOX], F32, kind="ExternalInput")
    oanchor = nc.dram_tensor("oanchor", [BOX, 6], F32, kind="ExternalInput")
    out = nc.dram_tensor("out", [1, 8], F32, kind="ExternalOutput")

    NEG = -1.0e9
    HB = 512  # columns per PSUM sim tile

    with tile.TileContext(nc) as tc:
        with tc.tile_pool(name="sb", bufs=1) as pool, \
             tc.tile_pool(name="ps1", bufs=2, space="PSUM") as psp1, \
             tc.tile_pool(name="ps", bufs=2, space="PSUM") as psp:
            # ---- loads: lhs first; quadrants split across sync+scalar HWDGE ----
            lhs = pool.tile([128, 2, BOX], F32)
            nc.sync.dma_start(out=lhs[:], in_=own_fnt.ap().rearrange("(h c) b -> c h b", h=2))
            src = fnt_all.ap().rearrange("(h c) j -> c h j", h=2)
            quad = {}
            engs = [nc.scalar, nc.sync, nc.scalar, nc.sync]
            i = 0
            for nb in range(2):
                for hh in range(2):
                    qt = pool.tile([128, HB], F32, tag=f"quad{hh}{nb}")
                    engs[i].dma_start(out=qt[:], in_=src[:, hh, nb * HB:(nb + 1) * HB])
                    i += 1
                    quad[(hh, nb)] = qt
            oanc = pool.tile([BOX, 6], F32)
            nc.scalar.dma_start(out=oanc[:], in_=oanchor.ap())

            # ---- sim in two PSUM tiles so nb=0 reductions overlap nb=1 matmuls ----
            sims = []
            for nb in range(2):
                s = psp1.tile([128, HB], F32, tag=f"sim{nb}")
                for hh in range(2):
                    nc.tensor.matmul(
                        out=s[:],
                        lhsT=lhs[:, hh, :],
                        rhs=quad[(hh, nb)][:],
                        start=(hh == 0),
                        stop=(hh == 1),
                    )
                sims.append(s)

            # ---- segment maxima per atom over sorted columns ----
            amax = pool.tile([BOX, 2, 3], F32)   # [s, c] triples
            if any(sz == 0 for sz in sizes):
                nc.vector.memset(amax[:], NEG)
            # issue all nb=0 segment reductions first (they only wait on sims[0])
            strad = {}
            for nb in range(2):
                for a in range(6):
                    lo, hi = offs[a], offs[a] + sizes[a]
                    l, h = max(lo, nb * HB), min(hi, (nb + 1) * HB)
                    if l >= h:
                        continue
                    dst = amax[:, a // 3, a % 3:a % 3 + 1]
                    seg = sims[nb][:, l - nb * HB:h - nb * HB]
                    if nb == 1 and a in strad:
                        t2 = pool.tile([BOX, 1], F32, tag=f"str{a}")
                        nc.vector.tensor_reduce(out=t2[:], in_=seg, op=ALU.max, axis=AX.X)
                        nc.vector.tensor_tensor(out=dst, in0=dst, in1=t2[:], op=ALU.max)
                    else:
                        nc.vector.tensor_reduce(out=dst, in_=seg, op=ALU.max, axis=AX.X)
                        strad[a] = True

            # ---- vectorized hinge over all 6 groups ----
            # P[:, s, c] = amax[:, 1-s, c] (opposite-state pos atom)
            P = pool.tile([BOX, 2, 3], F32)
            nc.vector.tensor_copy(out=P[:, 0, :], in_=amax[:, 1, :])
            nc.vector.tensor_copy(out=P[:, 1, :], in_=amax[:, 0, :])
            # N1[:, s, c] = P[:, s, (c+1)%3]; N2[:, s, c] = P[:, s, (c+2)%3]
            N1 = pool.tile([BOX, 2, 3], F32)
            nc.vector.tensor_copy(out=N1[:, :, 0:2], in_=P[:, :, 1:3])
            nc.vector.tensor_copy(out=N1[:, :, 2:3], in_=P[:, :, 0:1])
            N2 = pool.tile([BOX, 2, 3], F32)
            nc.vector.tensor_copy(out=N2[:, :, 0:1], in_=P[:, :, 2:3])
            nc.vector.tensor_copy(out=N2[:, :, 1:3], in_=P[:, :, 0:2])
            # hinge = relu((MARGIN + max(N1, N2)) - P), matching reference op order
            hx = pool.tile([BOX, 2, 3], F32)
            nc.vector.tensor_tensor(out=hx[:], in0=N1[:], in1=N2[:], op=ALU.max)
            nc.vector.tensor_scalar(out=hx[:], in0=hx[:], scalar1=float(MARGIN), scalar2=None, op0=ALU.add)
            nc.vector.tensor_tensor(out=hx[:], in0=hx[:], in1=P[:], op=ALU.subtract)
            nc.vector.tensor_scalar(out=hx[:], in0=hx[:], scalar1=0.0, scalar2=None, op0=ALU.max)
            rhs6 = pool.tile([BOX, 6], F32)
            nc.vector.tensor_tensor(out=rhs6[:], in0=hx[:].rearrange("p s c -> p (s c)"), in1=oanc[:], op=ALU.mult)

            ones = pool.tile([BOX, 1], F32)
            nc.vector.memset(ones[:], 1.0)
            psum_out = psp.tile([1, 6], F32, tag="po")
            nc.tensor.matmul(out=psum_out[:], lhsT=ones[:], rhs=rhs6[:], start=True, stop=True)
            osb = pool.tile([1, 8], F32)
            nc.vector.memset(osb[:], 0.0)
            nc.vector.tensor_copy(out=osb[:, 0:6], in_=psum_out[:])
            nc.sync.dma_start(out=out.ap(), in_=osb[:])
    nc.compile()
    return nc


_CACHE = {}
LAST_RESULTS = []   # [(phase, BassKernelResults), ...] of the most recent kernel() call


def _get_phase_a():
    if "a" not in _CACHE:
        _CACHE["a"] = build_phase_a()
    return _CACHE["a"]


def _get_phase_b(sizes):
    key = ("b", sizes)
    if key not in _CACHE:
        _CACHE[key] = build_phase_b(sizes)
    return _CACHE[key]


def kernel(spatial_features_2d: np.ndarray, gt_boxes: np.ndarray) -> np.ndarray:
    spatial = np.ascontiguousarray(spatial_features_2d, dtype=np.float32)
    boxes = np.ascontiguousarray(gt_boxes, dtype=np.float32)
    LAST_RESULTS.clear()

    # ---- host prep: channels-last planes (no box info used) ----
    planes = [np.ascontiguousarray(spatial[b].reshape(C, HW).T).reshape(-1) for b in range(B)]

    # ---- phase A: gather + normalize + transpose, data-parallel over boxes ----
    nca = _get_phase_a()
    # consts: one-hot diagonal mask (j == p//16) + the pixel ids 0..3
    consts = np.zeros((128, 12), dtype=np.float32)
    consts[:, 0:8] = np.arange(8)[None, :] == (np.arange(128) // 16)[:, None]
    consts[:, 8:12] = np.arange(4, dtype=np.float32)[None, :]
    in_a = []
    for k in range(NCORES):
        b = k // 2
        n0 = (k % 2) * BOX
        own_xy = boxes[b, n0:n0 + BOX, 0:2]
        # wrapped (j, q) -> box j*16+q at partition q, replicated to 8 groups
        wrapped = np.ascontiguousarray(own_xy.reshape(8, 16, 2).transpose(1, 0, 2))  # [16, 8, 2]
        rep = np.tile(wrapped, (8, 1, 1)).reshape(128, 8, 2)
        in_a.append({
            "spatial": planes[b],
            "boxes_rep": np.ascontiguousarray(rep),
            "consts": consts,
        })
    res_a = bass_utils.run_bass_kernel_spmd(nca, in_a, core_ids=list(range(NCORES)))
    LAST_RESULTS.append(("a", res_a))
    blocks = [res_a.results[k]["fnt"] for k in range(NCORES)]       # each [C, BOX]
    fnt_all = np.concatenate(blocks, axis=1)                        # [C, M]

    # ---- host: atom per box, sort columns by atom ----
    flag = boxes[..., 7].reshape(M)
    cls = boxes[..., 8].astype(np.int32).reshape(M)
    dyn = flag != 0
    atom = np.where(dyn, cls, 3 + cls).astype(np.int64)             # 0-2 dyn, 3-5 static
    perm = np.argsort(atom, kind="stable")
    sizes = tuple(int(x) for x in np.bincount(atom, minlength=6))
    fnt_sorted = np.ascontiguousarray(fnt_all[:, perm])
    atom_sorted = atom[perm]
    oanchor_full = (atom_sorted[:, None] == np.arange(6)[None, :]).astype(np.float32)

    # ---- phase B: sim block + segment maxima + partials ----
    ncb = _get_phase_b(sizes)
    in_b = []
    for k in range(NCORES):
        sl = slice(k * BOX, (k + 1) * BOX)
        in_b.append({
            "fnt_all": fnt_sorted,
            "own_fnt": np.ascontiguousarray(fnt_sorted[:, sl]),
            "oanchor": np.ascontiguousarray(oanchor_full[sl]),
        })
    res_b = bass_utils.run_bass_kernel_spmd(ncb, in_b, core_ids=list(range(NCORES)))
    LAST_RESULTS.append(("b", res_b))
    parts = np.stack([res_b.results[k]["out"][0] for k in range(NCORES)])  # [8, 8]

    # ---- host: assemble the scalar loss (f32, mirrors the reference) ----
    f32 = np.float32
    psums = parts[:, 0:6].astype(np.float32).sum(axis=0, dtype=np.float32)  # [6]
    total = f32(0.0)
    cnt = f32(0.0)
    for g in range(6):
        s_c = 0 if g >= 3 else 1
        c = g % 3
        n_a = sizes[g]
        n_pos = sizes[s_c * 3 + c]
        n_neg = sizes[s_c * 3 + (c + 1) % 3] + sizes[s_c * 3 + (c + 2) % 3]
        if (n_a > 0) and (n_pos > 0) and (n_neg > 0):
            total = f32(total + f32(psums[g] / f32(max(n_a, 1))))
            cnt = f32(cnt + 1.0)
    loss = f32(total / max(cnt, f32(1.0))) if cnt > 0 else f32(0.0)
    return np.asarray(loss, dtype=np.float32)


# revision 16
# speedup vs baseline: 1.4383x; 1.2601x over previous
"""DetContrastiveLoss Trainium2 kernel.

Two SPMD phases over 8 NeuronCores (no ncfw collectives — their entry
barrier + launch skew costs more than the 1MB exchange itself):

  Host prep: transpose each BEV plane to channels-last [H*W, C] so one
    box's 256 channel values are contiguous (4KB aligned rows). Pure
    layout prep — no box information used.

  Phase A (per core k): own 128 boxes of batch b=k//2. Compute box pixel
    index r = cy*W+cx on-device (exact f32 chain matching the CPU-jax
    reference truncation semantics), gather the 4-pixel-aligned block
    row r//4 (fits the int16 index limit: r//4 <= 32399) as 4 parallel
    dma_gathers (one per pixel offset, one SWDGE queue each, 1KB
    windows), select the r%4 pixel with 4 masked adds, L2-normalize
    rows (1/sqrt(temperature) folded in), transpose on PE -> [256, 128].

  Host: concat blocks -> fnT [256, 1024]; compute (state, class) atom
    per box from gt_boxes, sort columns by atom so per-atom masked
    maxima become segment maxima over static column slices. Group
    sizes are baked into the phase-B program at (lazy) compile time.

  Phase B (per core k): sim block [128, 1024] = own_fnT.T @ fnT_sorted
    via PE (f32, two 512-col PSUM tiles so column-0 reductions overlap
    the column-1 matmuls), 6 segment column maxima, vectorized hinge,
    anchor-masked column sums via PE -> [1, 8] per core.

  Host: assemble the scalar loss from 8x6 partials + atom counts
  (f32 arithmetic mirroring the reference).
"""

import sys

for _p in ("/opt/trn_rl_repo", "/root/.axon_site/_ro/trn_rl_repo"):
    if _p not in sys.path:
        sys.path.append(_p)

import numpy as np

import concourse.bass as bass
import concourse.bacc as bacc
import concourse.tile as tile
import concourse.mybir as mybir
from concourse import bass_utils
from concourse.masks import make_identity

F32 = mybir.dt.float32
BF16 = mybir.dt.bfloat16
I32 = mybir.dt.int32
I16 = mybir.dt.int16

B, N, C, H, W = 4, 256, 256, 360, 360
HW = H * W            # 129600
M = B * N             # 1024
NCORES = 8
BOX = 128             # boxes per core
TEMPERATURE = 0.1
MARGIN = 0.2
X0 = -59.9
SPAN = 119.8
PIX = 4               # pixels per aligned block
WIN = PIX * C         # floats per block (4KB)
NROWS = HW // PIX     # 32400 gather rows, fits int16
SQRT_INV_T = float(np.sqrt(np.float32(1.0) / np.float32(TEMPERATURE)))
INV_SPAN = float(np.float32(1.0) / np.float32(SPAN))

AX = mybir.AxisListType
ALU = mybir.AluOpType


def _floor(nc, pool, shape, t, tag):
    """Exact floor of non-negative f32 under any HW cast rounding."""
    ti = pool.tile(shape, I32, tag=f"{tag}_i")
    nc.vector.tensor_copy(out=ti[:], in_=t[:])
    tb = pool.tile(shape, F32, tag=f"{tag}_b")
    nc.vector.tensor_copy(out=tb[:], in_=ti[:])
    gt = pool.tile(shape, F32, tag=f"{tag}_g")
    nc.vector.tensor_tensor(out=gt[:], in0=tb[:], in1=t[:], op=ALU.is_gt)
    fl = pool.tile(shape, F32, tag=f"{tag}_f")
    nc.vector.tensor_tensor(out=fl[:], in0=tb[:], in1=gt[:], op=ALU.subtract)
    return fl


def _fused_chain(nc, pool, pshape, src_ap, tag):
    """src [..., 2] = (x, y) -> r = cy*W + cx, g = floor(r/4), both [..., 1].

    Chain matches CPU-jax truncation semantics: sub, mult-by-recip, mult,
    clip, floor (W == H so both coords share one fused pass).
    """
    t = pool.tile(pshape, F32, tag=f"t{tag}")
    nc.vector.tensor_scalar(out=t[:], in0=src_ap, scalar1=float(X0), scalar2=INV_SPAN, op0=ALU.subtract, op1=ALU.mult)
    nc.vector.tensor_scalar(out=t[:], in0=t[:], scalar1=float(W), scalar2=0.0, op0=ALU.mult, op1=ALU.max)
    nc.vector.tensor_scalar(out=t[:], in0=t[:], scalar1=float(W - 1), scalar2=None, op0=ALU.min)
    fl = _floor(nc, pool, pshape, t, tag)
    rshape = pshape[:-1] + [1]
    r = pool.tile(rshape, F32, tag=f"r{tag}")
    nc.vector.tensor_scalar(out=r[:], in0=fl[:, :, 1:2], scalar1=float(W), scalar2=None, op0=ALU.mult)
    nc.vector.tensor_tensor(out=r[:], in0=r[:], in1=fl[:, :, 0:1], op=ALU.add)
    gp = pool.tile(rshape, F32, tag=f"gp{tag}")
    nc.vector.tensor_scalar(out=gp[:], in0=r[:], scalar1=0.25, scalar2=None, op0=ALU.mult)
    g = _floor(nc, pool, rshape, gp, f"g{tag}")
    return r, g


def build_phase_a():
    nc = bacc.Bacc("TRN2", target_bir_lowering=False, debug=False, num_devices=NCORES)
    spatial = nc.dram_tensor("spatial", [HW * C], F32, kind="ExternalInput")  # channels-last
    boxes_rep = nc.dram_tensor("boxes_rep", [128, 8, 2], F32, kind="ExternalInput")  # wrapped+replicated xy
    consts = nc.dram_tensor("consts", [128, 12], F32, kind="ExternalInput")   # [dmask(8) | 0,1,2,3]
    fnt_out = nc.dram_tensor("fnt", [C, BOX], F32, kind="ExternalOutput")

    with tile.TileContext(nc) as tc:
        with tc.tile_pool(name="sb", bufs=1) as pool, \
             tc.tile_pool(name="ps", bufs=2, space="PSUM") as psp:
            # ---- load boxes (scalar) + host constants (sync) ----
            brep = pool.tile([128, 8, 2], F32)
            nc.scalar.dma_start(out=brep[:], in_=boxes_rep.ap())
            cst = pool.tile([128, 12], F32)
            nc.sync.dma_start(out=cst[:], in_=consts.ap())

            # ---- wrapped layout: gather row ids -> int16 (already replicated) ----
            r16, g16 = _fused_chain(nc, pool, [128, 8, 2], brep[:], "w")
            idx16 = pool.tile([128, 8], I16)
            nc.vector.tensor_copy(out=idx16[:], in_=g16[:].rearrange("p j f -> p (j f)"))

            # ---- within-block pixel offset o = r - 4g, extracted at the
            #      diagonal (partition p holds box p at slot j = p//16) ----
            o_rep = pool.tile([128, 8, 1], F32)
            nc.vector.tensor_scalar(out=o_rep[:], in0=g16[:], scalar1=-float(PIX), scalar2=None, op0=ALU.mult)
            nc.vector.tensor_tensor(out=o_rep[:], in0=r16[:], in1=o_rep[:], op=ALU.add)
            od = pool.tile([128, 8], F32)
            nc.vector.tensor_tensor(out=od[:], in0=o_rep[:].rearrange("p j f -> p (j f)"), in1=cst[:, 0:8], op=ALU.mult)
            o = pool.tile([BOX, 1], F32)
            nc.vector.tensor_reduce(out=o[:], in_=od[:], op=ALU.add, axis=AX.X)

            # ---- one gather: 128 windows of 4KB (4 pixels x 256 ch) ----
            win = pool.tile([128, 1, WIN], F32)
            nc.gpsimd.dma_gather(
                out_ap=win[:],
                in_ap=spatial.ap().rearrange("(r e) -> r e", e=WIN),
                idxs_ap=idx16[:],
                num_idxs=BOX,
                num_idxs_reg=BOX,
                elem_size=WIN,
                single_packet=False,
            )

            # ---- select the r%4 pixel: one-hot (o==q) mask, one mult + 3 adds ----
            eq4 = pool.tile([128, PIX, 1], F32)
            nc.vector.tensor_scalar(
                out=eq4[:],
                in0=cst[:, 8:12].rearrange("p (q f) -> p q f", f=1),
                scalar1=o[:], scalar2=None, op0=ALU.is_equal,
            )
            prod = pool.tile([128, PIX, C], F32)
            nc.vector.tensor_tensor(
                out=prod[:],
                in0=win[:, 0, :].rearrange("p (q c) -> p q c", q=PIX),
                in1=eq4[:].to_broadcast([128, PIX, C]),
                op=ALU.mult,
            )
            s01 = pool.tile([BOX, C], F32)
            nc.vector.tensor_tensor(out=s01[:], in0=prod[:, 0, :], in1=prod[:, 1, :], op=ALU.add)
            s23 = pool.tile([BOX, C], F32)
            nc.vector.tensor_tensor(out=s23[:], in0=prod[:, 2, :], in1=prod[:, 3, :], op=ALU.add)
            feats = pool.tile([BOX, C], F32)
            nc.vector.tensor_tensor(out=feats[:], in0=s01[:], in1=s23[:], op=ALU.add)

            # ---- normalize rows; fold 1/sqrt(T) ----
            sq = pool.tile([BOX, C], F32)
            nc.vector.tensor_tensor(out=sq[:], in0=feats[:], in1=feats[:], op=ALU.mult)
            ssq = pool.tile([BOX, 1], F32)
            nc.vector.tensor_reduce(out=ssq[:], in_=sq[:], op=ALU.add, axis=AX.X)
            nc.vector.tensor_scalar(out=ssq[:], in0=ssq[:], scalar1=1e-24, scalar2=None, op0=ALU.max)
            rt = pool.tile([BOX, 1], F32)
            nc.vector.reciprocal(out=rt[:], in_=ssq[:])          # 1/ssq
            nc.scalar.activation(rt[:], rt[:], mybir.ActivationFunctionType.Sqrt)  # ~1/norm
            # one Newton step on r ~= rsqrt(ssq): r' = r*(1.5 - 0.5*ssq*r^2)
            r2 = pool.tile([BOX, 1], F32)
            nc.vector.tensor_tensor(out=r2[:], in0=rt[:], in1=rt[:], op=ALU.mult)
            nc.vector.tensor_tensor(out=r2[:], in0=r2[:], in1=ssq[:], op=ALU.mult)
            nc.vector.tensor_scalar(out=r2[:], in0=r2[:], scalar1=-0.5, scalar2=1.5, op0=ALU.mult, op1=ALU.add)
            nc.vector.tensor_tensor(out=rt[:], in0=rt[:], in1=r2[:], op=ALU.mult)
            nc.vector.tensor_scalar(out=rt[:], in0=rt[:], scalar1=SQRT_INV_T, scalar2=None, op0=ALU.mult)
            fn = pool.tile([BOX, C], F32)
            nc.vector.tensor_scalar(out=fn[:], in0=feats[:], scalar1=rt[:], scalar2=None, op0=ALU.mult)

            # ---- transpose [128, 256] -> [256, 128] via PE ----
            ident = pool.tile([128, 128], F32)
            make_identity(nc, ident[:])
            fnt_sb = pool.tile([128, 2, 128], F32)
            for hh in range(2):
                pst = psp.tile([128, 128], F32, tag="pst")
                nc.tensor.transpose(out=pst[:], in_=fn[:, hh * 128:(hh + 1) * 128], identity=ident[:])
                nc.vector.tensor_copy(out=fnt_sb[:, hh, :], in_=pst[:])
            nc.sync.dma_start(
                out=fnt_out.ap().rearrange("(h c) b -> c h b", h=2),
                in_=fnt_sb[:],
            )
    nc.compile()
    return nc


def build_phase_b(sizes):
    """sizes: tuple of 6 ints (sorted atom group sizes, sum == M)."""
    offs = [0] * 6
    for a in range(1, 6):
        offs[a] = offs[a - 1] + sizes[a - 1]

    nc = bacc.Bacc("TRN2", target_bir_lowering=False, debug=False, num_devices=NCORES)
    # bf16 features: host-validated loss shift 7e-5 rel (gate 2e-2); sim
    # accumulates in f32 PSUM. Halves both PE passes and load traffic.
    fnt_all = nc.dram_tensor("fnt_all", [C, M], BF16, kind="ExternalInput")
    own_fnt = nc.dram_tensor("own_fnt", [C, BOX], BF16, kind="ExternalInput")
    oanchor = nc.dram_tensor("oanchor", [BOX, 6], F32, kind="ExternalInput")
    out = nc.dram_tensor("out", [1, 8], F32, kind="ExternalOutput")

    NEG = -1.0e9
    HB = 512  # columns per PSUM sim tile

    with tile.TileContext(nc) as tc:
        with tc.tile_pool(name="sb", bufs=1) as pool, \
             tc.tile_pool(name="ps1", bufs=2, space="PSUM") as psp1, \
             tc.tile_pool(name="ps", bufs=2, space="PSUM") as psp:
            # ---- loads: lhs first; quadrants split across sync+scalar HWDGE ----
            lhs = pool.tile([128, 2, BOX], BF16)
            nc.sync.dma_start(out=lhs[:], in_=own_fnt.ap().rearrange("(h c) b -> c h b", h=2))
            src = fnt_all.ap().rearrange("(h c) j -> c h j", h=2)
            quad = {}
            engs = [nc.scalar, nc.sync, nc.scalar, nc.sync]
            i = 0
            for nb in range(2):
                for hh in range(2):
                    qt = pool.tile([128, HB], BF16, tag=f"quad{hh}{nb}")
                    engs[i].dma_start(out=qt[:], in_=src[:, hh, nb * HB:(nb + 1) * HB])
                    i += 1
                    quad[(hh, nb)] = qt
            oanc = pool.tile([BOX, 6], F32)
            nc.scalar.dma_start(out=oanc[:], in_=oanchor.ap())

            # ---- sim in two PSUM tiles so nb=0 reductions overlap nb=1 matmuls ----
            sims = []
            for nb in range(2):
                s = psp1.tile([128, HB], F32, tag=f"sim{nb}")
                for hh in range(2):
                    nc.tensor.matmul(
                        out=s[:],
                        lhsT=lhs[:, hh, :],
                        rhs=quad[(hh, nb)][:],
                        start=(hh == 0),
                        stop=(hh == 1),
                    )
                sims.append(s)

            # ---- segment maxima per atom over sorted columns ----
            amax = pool.tile([BOX, 2, 3], F32)   # [s, c] triples
            if any(sz == 0 for sz in sizes):
                nc.vector.memset(amax[:], NEG)
            # issue all nb=0 segment reductions first (they only wait on sims[0])
            strad = {}
            for nb in range(2):
                for a in range(6):
                    lo, hi = offs[a], offs[a] + sizes[a]
                    l, h = max(lo, nb * HB), min(hi, (nb + 1) * HB)
                    if l >= h:
                        continue
                    dst = amax[:, a // 3, a % 3:a % 3 + 1]
                    seg = sims[nb][:, l - nb * HB:h - nb * HB]
                    if nb == 1 and a in strad:
                        t2 = pool.tile([BOX, 1], F32, tag=f"str{a}")
                        nc.vector.tensor_reduce(out=t2[:], in_=seg, op=ALU.max, axis=AX.X)
                        nc.vector.tensor_tensor(out=dst, in0=dst, in1=t2[:], op=ALU.max)
                    else:
                        nc.vector.tensor_reduce(out=dst, in_=seg, op=ALU.max, axis=AX.X)
                        strad[a] = True

            # ---- vectorized hinge over all 6 groups ----
            # P[:, s, c] = amax[:, 1-s, c] (opposite-state pos atom)
            P = pool.tile([BOX, 2, 3], F32)
            nc.vector.tensor_copy(out=P[:, 0, :], in_=amax[:, 1, :])
            nc.vector.tensor_copy(out=P[:, 1, :], in_=amax[:, 0, :])
            # N1[:, s, c] = P[:, s, (c+1)%3]; N2[:, s, c] = P[:, s, (c+2)%3]
            N1 = pool.tile([BOX, 2, 3], F32)
            nc.vector.tensor_copy(out=N1[:, :, 0:2], in_=P[:, :, 1:3])
            nc.vector.tensor_copy(out=N1[:, :, 2:3], in_=P[:, :, 0:1])
            N2 = pool.tile([BOX, 2, 3], F32)
            nc.vector.tensor_copy(out=N2[:, :, 0:1], in_=P[:, :, 2:3])
            nc.vector.tensor_copy(out=N2[:, :, 1:3], in_=P[:, :, 0:2])
            # hinge = relu((MARGIN + max(N1, N2)) - P), matching reference op order
            hx = pool.tile([BOX, 2, 3], F32)
            nc.vector.tensor_tensor(out=hx[:], in0=N1[:], in1=N2[:], op=ALU.max)
            nc.vector.tensor_scalar(out=hx[:], in0=hx[:], scalar1=float(MARGIN), scalar2=None, op0=ALU.add)
            nc.vector.tensor_tensor(out=hx[:], in0=hx[:], in1=P[:], op=ALU.subtract)
            nc.vector.tensor_scalar(out=hx[:], in0=hx[:], scalar1=0.0, scalar2=None, op0=ALU.max)
            rhs6 = pool.tile([BOX, 6], F32)
            nc.vector.tensor_tensor(out=rhs6[:], in0=hx[:].rearrange("p s c -> p (s c)"), in1=oanc[:], op=ALU.mult)

            ones = pool.tile([BOX, 1], F32)
            nc.vector.memset(ones[:], 1.0)
            psum_out = psp.tile([1, 6], F32, tag="po")
            nc.tensor.matmul(out=psum_out[:], lhsT=ones[:], rhs=rhs6[:], start=True, stop=True)
            osb = pool.tile([1, 8], F32)
            nc.vector.memset(osb[:], 0.0)
            nc.vector.tensor_copy(out=osb[:, 0:6], in_=psum_out[:])
            nc.sync.dma_start(out=out.ap(), in_=osb[:])
    nc.compile()
    return nc


_CACHE = {}
LAST_RESULTS = []   # [(phase, BassKernelResults), ...] of the most recent kernel() call


def _get_phase_a():
    if "a" not in _CACHE:
        _CACHE["a"] = build_phase_a()
    return _CACHE["a"]


def _get_phase_b(sizes):
    key = ("b", sizes)
    if key not in _CACHE:
        _CACHE[key] = build_phase_b(sizes)
    return _CACHE[key]


def kernel(spatial_features_2d: np.ndarray, gt_boxes: np.ndarray) -> np.ndarray:
    spatial = np.ascontiguousarray(spatial_features_2d, dtype=np.float32)
    boxes = np.ascontiguousarray(gt_boxes, dtype=np.float32)
    LAST_RESULTS.clear()

    # ---- host prep: channels-last planes (no box info used) ----
    planes = [np.ascontiguousarray(spatial[b].reshape(C, HW).T).reshape(-1) for b in range(B)]

    # ---- phase A: gather + normalize + transpose, data-parallel over boxes ----
    nca = _get_phase_a()
    # consts: one-hot diagonal mask (j == p//16) + the pixel ids 0..3
    consts = np.zeros((128, 12), dtype=np.float32)
    consts[:, 0:8] = np.arange(8)[None, :] == (np.arange(128) // 16)[:, None]
    consts[:, 8:12] = np.arange(4, dtype=np.float32)[None, :]
    in_a = []
    for k in range(NCORES):
        b = k // 2
        n0 = (k % 2) * BOX
        own_xy = boxes[b, n0:n0 + BOX, 0:2]
        # wrapped (j, q) -> box j*16+q at partition q, replicated to 8 groups
        wrapped = np.ascontiguousarray(own_xy.reshape(8, 16, 2).transpose(1, 0, 2))  # [16, 8, 2]
        rep = np.tile(wrapped, (8, 1, 1)).reshape(128, 8, 2)
        in_a.append({
            "spatial": planes[b],
            "boxes_rep": np.ascontiguousarray(rep),
            "consts": consts,
        })
    res_a = bass_utils.run_bass_kernel_spmd(nca, in_a, core_ids=list(range(NCORES)))
    LAST_RESULTS.append(("a", res_a))
    blocks = [res_a.results[k]["fnt"] for k in range(NCORES)]       # each [C, BOX]
    fnt_all = np.concatenate(blocks, axis=1)                        # [C, M]

    # ---- host: atom per box, sort columns by atom ----
    flag = boxes[..., 7].reshape(M)
    cls = boxes[..., 8].astype(np.int32).reshape(M)
    dyn = flag != 0
    atom = np.where(dyn, cls, 3 + cls).astype(np.int64)             # 0-2 dyn, 3-5 static
    perm = np.argsort(atom, kind="stable")
    sizes = tuple(int(x) for x in np.bincount(atom, minlength=6))
    import ml_dtypes
    fnt_sorted = np.ascontiguousarray(fnt_all[:, perm].astype(ml_dtypes.bfloat16))
    atom_sorted = atom[perm]
    oanchor_full = (atom_sorted[:, None] == np.arange(6)[None, :]).astype(np.float32)

    # ---- phase B: sim block + segment maxima + partials ----
    ncb = _get_phase_b(sizes)
    in_b = []
    for k in range(NCORES):
        sl = slice(k * BOX, (k + 1) * BOX)
        in_b.append({
            "fnt_all": fnt_sorted,
            "own_fnt": np.ascontiguousarray(fnt_sorted[:, sl]),
            "oanchor": np.ascontiguousarray(oanchor_full[sl]),
        })
    res_b = bass_utils.run_bass_kernel_spmd(ncb, in_b, core_ids=list(range(NCORES)))
    LAST_RESULTS.append(("b", res_b))
    parts = np.stack([res_b.results[k]["out"][0] for k in range(NCORES)])  # [8, 8]

    # ---- host: assemble the scalar loss (f32, mirrors the reference) ----
    f32 = np.float32
    psums = parts[:, 0:6].astype(np.float32).sum(axis=0, dtype=np.float32)  # [6]
    total = f32(0.0)
    cnt = f32(0.0)
    for g in range(6):
        s_c = 0 if g >= 3 else 1
        c = g % 3
        n_a = sizes[g]
        n_pos = sizes[s_c * 3 + c]
        n_neg = sizes[s_c * 3 + (c + 1) % 3] + sizes[s_c * 3 + (c + 2) % 3]
        if (n_a > 0) and (n_pos > 0) and (n_neg > 0):
            total = f32(total + f32(psums[g] / f32(max(n_a, 1))))
            cnt = f32(cnt + 1.0)
    loss = f32(total / max(cnt, f32(1.0))) if cnt > 0 else f32(0.0)
    return np.asarray(loss, dtype=np.float32)
